# revision 35
# baseline (speedup 1.0000x reference)
"""MultiHeadAttention TRN2 Bass kernel, sharded over 8 NeuronCores.

Sharding: 8 cores = 2 batches x 4 head-groups. Each core computes 4 heads of
one batch end-to-end (q/k/v projections, biased+masked softmax attention, and
a partial output projection); the host sums the per-group partial outputs.

On-device layout is fully "transposed" so no on-device transposes are needed:
  - host supplies x^T [D, S] per batch (bf16) and per-core weight slices
  - projections produce qT/kT [head_dims, S]; v stays natural [S, head_dims]
  - scores are computed transposed: scoresT[s_k, s_q] = kT.T @ qT per head,
    with the other head's kT rows zeroed so every matmul contracts K=128
  - softmax: exp on ScalarE (PSUM->SBUF), bias/mask applied as a multiply
    with host-precomputed exp(bias_masked)^T on VectorE, and the denominator
    comes free as an extra ones-column in the attn@v matmul
  - attn@v: out2[dh+1, s_q] accumulated over s_k chunks; normalization by the
    ones-row + per-head v-bias correction happens on the way into the concat
    tile; output projection emits partial_out^T [D, S] (f32) per core.
"""

import numpy as np
import ml_dtypes

import concourse.bass as bass
import concourse.mybir as mybir
import concourse.tile as tile
from concourse.bacc import Bacc

BF16 = mybir.dt.bfloat16
F32 = mybir.dt.float32
nbf16 = ml_dtypes.bfloat16

B = 2
S_FULL = 2048
D = 1024
H = 16
DH = 64
HPC = 4  # heads per core
CD = HPC * DH  # 256 per-core projected dims
NCORES = 8
SCALE = 8.0  # sqrt(DH)

KC = D // 128  # 8 contraction chunks for projections
NB = 512  # projection token-block (free dim per matmul)


def build_module(S=S_FULL, debug=False):
    """Build the single-core Bass program (same program runs SPMD on 8 cores)."""
    assert S % 1024 == 0
    SUPS = 2  # s_q superblocks
    SUPLEN = S // SUPS  # columns per superblock
    NHALF = SUPLEN // NB  # matmuls per psum row-tile
    NT = S // NB  # projection token blocks
    TC = S // 128  # token / s_k chunks

    # Bacc (not plain Bass): its compile() splits multi-wait instructions to
    # the 1-wait HW limit and inserts library/ACT-table loads, which the
    # neuronxcc walrus codegen path requires.
    nc = Bacc(None)

    xqT = nc.dram_tensor("xqT", [D, S], BF16, kind="ExternalInput")
    xkT = nc.dram_tensor("xkT", [D, S], BF16, kind="ExternalInput")
    xvT = nc.dram_tensor("xvT", [D, S], BF16, kind="ExternalInput")
    # weights arrive pre-arranged [128, kc*CD] so the load is one fully
    # contiguous-per-partition DMA
    wqT = nc.dram_tensor("wqT", [128, KC * CD], BF16, kind="ExternalInput")
    wkT = nc.dram_tensor("wkT", [128, KC * CD], BF16, kind="ExternalInput")
    wvT = nc.dram_tensor("wvT", [128, KC * CD], BF16, kind="ExternalInput")
    woT = nc.dram_tensor("woT", [128, (CD // 128) * D], BF16, kind="ExternalInput")
    bqc = nc.dram_tensor("bqc", [128, 2], F32, kind="ExternalInput")
    bkc = nc.dram_tensor("bkc", [128, 2], F32, kind="ExternalInput")
    bvc = nc.dram_tensor("bvc", [64, HPC], F32, kind="ExternalInput")
    expbT = nc.dram_tensor("expbT", [S, S], BF16, kind="ExternalInput")
    poutT = nc.dram_tensor("poutT", [D, S], F32, kind="ExternalOutput")
    if debug:
        TCD = S // 128
        dbg_qT = nc.dram_tensor("dbg_qT", [2, 128, S], BF16, kind="ExternalOutput")
        dbg_kTh = nc.dram_tensor("dbg_kTh", [HPC, 128, S], BF16, kind="ExternalOutput")
        dbg_vv = nc.dram_tensor(
            "dbg_vv", [128, HPC * TCD * (DH + 1)], BF16, kind="ExternalOutput"
        )
        dbg_cc = nc.dram_tensor("dbg_cc", [2, 128, S], BF16, kind="ExternalOutput")
        dbg_rb = nc.dram_tensor("dbg_rb", [2, HPC, 64, S // 2], F32, kind="ExternalOutput")
        dbg_ea = nc.dram_tensor("dbg_ea", [2, 128, S // 2], BF16, kind="ExternalOutput")

    with tile.TileContext(nc) as tc:
        with (
            tc.tile_pool(name="statics", bufs=1) as statics,
            tc.tile_pool(name="xs", bufs=12) as xs_pool,
            tc.tile_pool(name="xv", bufs=KC) as xv_pool,
            tc.tile_pool(name="expb", bufs=2) as expb_pool,
            tc.tile_pool(name="e", bufs=4) as e_pool,
            tc.tile_pool(name="a", bufs=4) as a_pool,
            tc.tile_pool(name="rec", bufs=2) as rec_pool,
            tc.tile_pool(name="spr", bufs=2) as spread_pool,
            tc.tile_pool(name="rb", bufs=2) as rb_pool,
            tc.tile_pool(name="segt", bufs=2) as seg_pool,
            tc.tile_pool(name="oev", bufs=3) as oev_pool,
            tc.tile_pool(name="psc", bufs=2, space="PSUM") as psc,
            tc.tile_pool(name="pacc", bufs=2, space="PSUM") as pacc,
            tc.tile_pool(name="dsc", bufs=4, space="DRAM") as dram_pool,
        ):
            # ---- static tiles ----
            wq_sb = statics.tile([128, KC, CD], BF16, name="wq_sb")
            wk_sb = statics.tile([128, KC, CD], BF16, name="wk_sb")
            wv_sb = statics.tile([128, KC, CD], BF16, name="wv_sb")
            wo_sb = statics.tile([128, CD // 128, D], BF16, name="wo_sb")
            bq_sb = statics.tile([128, 2], F32, name="bq_sb")
            bk_sb = statics.tile([128, 2], F32, name="bk_sb")
            bv_sb = statics.tile([64, HPC], F32, name="bv_sb")
            qT = [statics.tile([128, S], BF16, name=f"qT{m}") for m in range(2)]
            # per-head kT, zero-padded on the other head's 64 rows so scores
            # matmuls contract a full K=128
            kTh = [statics.tile([128, S], BF16, name=f"kTh{h}") for h in range(HPC)]
            vv = statics.tile([128, HPC, TC, DH + 1], BF16, name="vv")
            cc = [statics.tile([128, S], BF16, name=f"cc{m}") for m in range(2)]

            nc.sync.dma_start(wq_sb, wqT[:, :].rearrange("p (kc m) -> p kc m", kc=KC))
            nc.sync.dma_start(wk_sb, wkT[:, :].rearrange("p (kc m) -> p kc m", kc=KC))
            nc.sync.dma_start(wv_sb, wvT[:, :].rearrange("p (kc m) -> p kc m", kc=KC))
            nc.sync.dma_start(
                wo_sb, woT[:, :].rearrange("p (kc m) -> p kc m", kc=CD // 128)
            )
            nc.sync.dma_start(bq_sb, bqc[:, :])
            nc.sync.dma_start(bk_sb, bkc[:, :])
            nc.sync.dma_start(bv_sb, bvc[:, :])

            # zero-fills on the otherwise-idle GpSimd engine
            for h in range(HPC):
                nc.gpsimd.memset(kTh[h], 0.0)
            nc.gpsimd.memset(vv[:, :, :, DH : DH + 1], 1.0)

            # ---- phase 1: q/k projections (transposed outputs) ----
            for xdram, w_sb, b_sb, is_q in (
                (xqT, wq_sb, bq_sb, True),
                (xkT, wk_sb, bk_sb, False),
            ):
                for nt in range(NT):
                    xts = []
                    for kc in range(KC):
                        xt = xs_pool.tile([128, NB], BF16, name="xt")
                        nc.sync.dma_start(
                            xt,
                            xdram[kc * 128 : (kc + 1) * 128, nt * NB : (nt + 1) * NB],
                        )
                        xts.append(xt)
                    for mt in range(2):
                        ps = psc.tile([128, NB], F32, name="ps_proj", tag="psc")
                        for kc in range(KC):
                            nc.tensor.matmul(
                                ps,
                                lhsT=w_sb[:, kc, mt * 128 : (mt + 1) * 128],
                                rhs=xts[kc],
                                start=(kc == 0),
                                stop=(kc == KC - 1),
                            )
                        # evacuate on DVE (tensor_scalar add with per-partition
                        # bias) to keep ScalarE free for the exp stream
                        csl = slice(nt * NB, (nt + 1) * NB)
                        if is_q:
                            nc.vector.tensor_scalar_add(
                                qT[mt][:, csl], ps, scalar1=b_sb[:, mt : mt + 1]
                            )
                        else:
                            # split the head-pair psum into the two zero-padded
                            # per-head kT tiles (lane-aligned halves)
                            h0, h1 = 2 * mt, 2 * mt + 1
                            nc.vector.tensor_scalar_add(
                                kTh[h0][0:64, csl],
                                ps[0:64, :],
                                scalar1=b_sb[0:64, mt : mt + 1],
                            )
                            nc.vector.tensor_scalar_add(
                                kTh[h1][64:128, csl],
                                ps[64:128, :],
                                scalar1=b_sb[64:128, mt : mt + 1],
                            )

            # ---- phase 1b: v projection (natural layout, no bias) ----
            xv_tiles = []
            for kc in range(KC):
                xt = xv_pool.tile([128, S], BF16, name="xvt")
                nc.sync.dma_start(xt, xvT[kc * 128 : (kc + 1) * 128, :])
                xv_tiles.append(xt)

            # exp(bias_masked)^T superblocks — emitted after ALL projection
            # x loads so those win the DMA queues at kernel start; the
            # 4MB/superblock transfers still finish before attention needs them
            expb_tiles = []
            for sup in range(SUPS):
                t = expb_pool.tile([128, TC, SUPLEN], BF16, name="expb")
                nc.sync.dma_start(
                    t,
                    expbT[:, sup * SUPLEN : (sup + 1) * SUPLEN].rearrange(
                        "(c p) q -> p c q", p=128
                    ),
                )
                expb_tiles.append(t)
            for tk in range(TC):
                ps = pacc.tile([128, CD], F32, name="ps_v", tag="pacc")
                for kc in range(KC):
                    nc.tensor.matmul(
                        ps,
                        lhsT=xv_tiles[kc][:, tk * 128 : (tk + 1) * 128],
                        rhs=wv_sb[:, kc, :],
                        start=(kc == 0),
                        stop=(kc == KC - 1),
                    )
                nc.vector.tensor_copy(
                    vv[:, :, tk, 0:DH],
                    ps.rearrange("p (h d) -> p h d", h=HPC),
                )

            # ---- phase 2: attention ----
            # The epilogue (normalize-by-sum) of instance i-1 is software-
            # pipelined into instance i's chunk loop in three stages so the
            # reciprocal/broadcast DMA chain never stalls the in-order DVE
            # stream that feeds PE with A tiles.
            def make_epilogue(sup, h, out2):
                qsl = slice(sup * SUPLEN, (sup + 1) * SUPLEN)
                mt = h // 2
                st = {}

                def s1():
                    # sum row PSUM->SBUF, then spread the 1xN row across 128
                    # partitions via DRAM so the reciprocal runs wide
                    st["ssum"] = rec_pool.tile([DH + 1, SUPLEN], F32, name="ssum")
                    nc.vector.tensor_copy(
                        st["ssum"][DH : DH + 1, :], out2[DH : DH + 1, :]
                    )
                    st["rsd"] = dram_pool.tile([1, SUPLEN], F32, name="rsd")
                    nc.sync.dma_start(st["rsd"], st["ssum"][DH : DH + 1, :])
                    st["spread"] = spread_pool.tile([128, SUPLEN // 128], F32, name="spread")
                    nc.sync.dma_start(
                        st["spread"],
                        st["rsd"][:, :].rearrange("a (p f) -> (a p) f", p=128),
                    )

                def s2():
                    nc.vector.reciprocal(st["spread"], st["spread"])
                    st["rsd2"] = dram_pool.tile([1, SUPLEN], F32, name="rsd2")
                    nc.sync.dma_start(
                        st["rsd2"][:, :].rearrange("a (p f) -> (a p) f", p=128),
                        st["spread"],
                    )
                    st["rb"] = rb_pool.tile([64, SUPLEN], F32, name="rb")
                    nc.sync.dma_start(
                        st["rb"], st["rsd2"][:, :].partition_broadcast(64)
                    )

                def s3():
                    rb = st["rb"]
                    if debug:
                        nc.sync.dma_start(dbg_rb[sup, h, :, :], rb)
                    if h % 2 == 0:
                        seg = cc[mt][0:64, qsl]
                        nc.vector.tensor_mul(seg, out2[0:DH, :], rb)
                        nc.vector.tensor_scalar_add(
                            seg, seg, scalar1=bv_sb[:, h : h + 1]
                        )
                    else:
                        segt = seg_pool.tile([64, SUPLEN], BF16, name="segt")
                        nc.vector.tensor_mul(segt, out2[0:DH, :], rb)
                        nc.vector.tensor_scalar_add(
                            segt, segt, scalar1=bv_sb[:, h : h + 1]
                        )
                        # partition move 0-63 -> 64-127 via DMA
                        nc.sync.dma_start(cc[mt][64:128, qsl], segt)

                return (s1, s2, s3)

            pending = None
            for sup in range(SUPS):
                for h in range(HPC):
                    mt = h // 2
                    out2 = pacc.tile([DH + 1, SUPLEN], F32, name="out2", tag="pacc")
                    for ck in range(TC):
                        sc = psc.tile([128, SUPLEN], F32, name="sc", tag="psc")
                        lhsT_k = kTh[h][:, ck * 128 : (ck + 1) * 128]
                        for hf in range(NHALF):
                            hsl = slice(hf * NB, (hf + 1) * NB)
                            nc.tensor.matmul(
                                sc[:, hsl],
                                lhsT=lhsT_k,
                                rhs=qT[mt][:, sup * SUPLEN + hf * NB : sup * SUPLEN + (hf + 1) * NB],
                                start=True,
                                stop=True,
                            )
                        e = e_pool.tile([128, SUPLEN], BF16, name="e")
                        nc.scalar.activation(
                            e, sc, func=mybir.ActivationFunctionType.Exp
                        )
                        a = a_pool.tile([128, SUPLEN], BF16, name="a")
                        nc.vector.tensor_mul(a, e, expb_tiles[sup][:, ck, :])
                        if debug and sup == 0 and h == 0 and ck == 0:
                            nc.sync.dma_start(dbg_ea[0, :, :], e)
                            nc.sync.dma_start(dbg_ea[1, :, :], a)
                        for hf in range(NHALF):
                            hsl = slice(hf * NB, (hf + 1) * NB)
                            nc.tensor.matmul(
                                out2[:, hsl],
                                lhsT=vv[:, h, ck, :],
                                rhs=a[:, hsl],
                                start=(ck == 0),
                                stop=(ck == TC - 1),
                            )
                        if pending is not None:
                            if ck == 0:
                                pending[0]()
                            elif ck == TC // 4:
                                pending[1]()
                            elif ck == TC // 2:
                                pending[2]()
                    pending = make_epilogue(sup, h, out2)
            if debug:
                for stage in pending:
                    stage()
                pending = None
                for m in range(2):
                    nc.sync.dma_start(dbg_qT[m, :, :], qT[m])
                    nc.sync.dma_start(dbg_cc[m, :, :], cc[m])
                for h in range(HPC):
                    nc.sync.dma_start(dbg_kTh[h, :, :], kTh[h])
                nc.sync.dma_start(dbg_vv[:, :], vv.rearrange("p a b c -> p (a b c)"))

            # ---- phase 3: output projection (transposed partial output) ----
            # nt order puts sup-0 token blocks first and interleaves the final
            # attention instance's epilogue stages into them, so PE never
            # idles waiting for the last normalize. Uses the (now idle)
            # scores PSUM slots for double-buffering.
            def outproj_tile(mo, nt):
                # alternate between the two PSUM pools for 4-deep pipelining
                pool = psc if (mo * NT + nt) % 2 == 0 else pacc
                tag = "psc" if (mo * NT + nt) % 2 == 0 else "pacc"
                ps = pool.tile([128, NB], F32, name="ps_o", tag=tag)
                for kc in range(CD // 128):
                    nc.tensor.matmul(
                        ps,
                        lhsT=wo_sb[:, kc, mo * 128 : (mo + 1) * 128],
                        rhs=cc[kc][:, nt * NB : (nt + 1) * NB],
                        start=(kc == 0),
                        stop=(kc == CD // 128 - 1),
                    )
                ot = oev_pool.tile([128, NB], F32, name="ot")
                # alternate evacuation between DVE and ACT to halve the
                # tail-phase copy bottleneck
                if (mo * NT + nt) % 2 == 0:
                    nc.vector.tensor_copy(ot, ps)
                else:
                    nc.scalar.copy(ot, ps)
                nc.sync.dma_start(
                    poutT[mo * 128 : (mo + 1) * 128, nt * NB : (nt + 1) * NB], ot
                )

            sup0_nts = list(range(NT // 2))
            sup1_nts = list(range(NT // 2, NT))
            emitted = 0
            stages_done = 0
            for nt in sup0_nts:
                for mo in range(D // 128):
                    outproj_tile(mo, nt)
                    emitted += 1
                    if pending is not None and stages_done < 3 and emitted % 4 == 0:
                        pending[stages_done]()
                        stages_done += 1
            if pending is not None:
                while stages_done < 3:
                    pending[stages_done]()
                    stages_done += 1
                pending = None
            for nt in sup1_nts:
                for mo in range(D // 128):
                    outproj_tile(mo, nt)

    nc.finalize()  # runs Bacc.compile(): wait-splitting, reg alloc, table loads
    return nc


def make_in_maps(query, key, value, mask, chemical_bias, Wq, bq, Wk, bk, Wv, bv, Wo, S=S_FULL):
    """Host-side preprocessing: per-core input dicts (8 cores)."""
    f32 = np.float32

    def c(a, dt):
        return np.ascontiguousarray(a, dtype=dt)

    per_batch = []
    for b in range(B):
        xq = c(query[b].T, nbf16)
        xk = c(key[b].T, nbf16)
        xv = c(value[b].T, nbf16)
        bm = np.where(mask[b, 0] == 0, f32(0.0), np.exp(chemical_bias[b], dtype=f32))
        expbT_ = c(bm.T, nbf16)
        per_batch.append((xq, xk, xv, expbT_))

    def warr(wt, kc):
        # [kc*128, M] -> [128, kc*M]: per-partition-contiguous device layout
        m = wt.shape[1]
        return np.ascontiguousarray(
            wt.reshape(kc, 128, m).transpose(1, 0, 2).reshape(128, kc * m), nbf16
        )

    per_group = []
    for g in range(4):
        hsl = slice(g * CD, (g + 1) * CD)
        wqT_ = warr(np.asarray((Wq[hsl] / SCALE).T, np.float32), KC)
        wkT_ = warr(np.asarray(Wk[hsl].T, np.float32), KC)
        wvT_ = warr(np.asarray(Wv[hsl].T, np.float32), KC)
        woT_ = warr(np.asarray(Wo[:, hsl].T, np.float32), CD // 128)
        bqc_ = c((bq[hsl] / SCALE).reshape(2, 128).T, f32)
        bkc_ = c(bk[hsl].reshape(2, 128).T, f32)
        bvc_ = c(bv[hsl].reshape(HPC, 64).T, f32)
        per_group.append((wqT_, wkT_, wvT_, woT_, bqc_, bkc_, bvc_))

    in_maps = []
    for core in range(NCORES):
        b, g = divmod(core, 4)
        xq, xk, xv, expbT_ = per_batch[b]
        wqT_, wkT_, wvT_, woT_, bqc_, bkc_, bvc_ = per_group[g]
        in_maps.append(
            {
                "xqT": xq,
                "xkT": xk,
                "xvT": xv,
                "wqT": wqT_,
                "wkT": wkT_,
                "wvT": wvT_,
                "woT": woT_,
                "bqc": bqc_,
                "bkc": bkc_,
                "bvc": bvc_,
                "expbT": expbT_,
            }
        )
    return in_maps


def combine_outputs(results, bo):
    """Sum per-group transposed partials into the full [B, S, D] output."""
    out = np.empty((B, S_FULL, D), np.float32)
    for b in range(B):
        acc = results[4 * b]["poutT"].T.astype(np.float32).copy()
        for g in range(1, 4):
            acc += results[4 * b + g]["poutT"].T
        out[b] = acc + bo.astype(np.float32)
    return out


_NC_CACHE = {}


def _get_module(S=S_FULL, debug=False):
    key = (S, debug)
    if key not in _NC_CACHE:
        _NC_CACHE[key] = build_module(S, debug=debug)
    return _NC_CACHE[key]


def run_spmd(in_maps, S=S_FULL, debug=False, **kwargs):
    from concourse.bass_utils import run_bass_kernel_spmd

    nc = _get_module(S, debug)
    return run_bass_kernel_spmd(nc, in_maps, core_ids=list(range(NCORES)), **kwargs)


def kernel(query, key, value, mask, chemical_bias, Wq, bq, Wk, bk, Wv, bv, Wo, bo):
    in_maps = make_in_maps(
        query, key, value, mask, chemical_bias, Wq, bq, Wk, bk, Wv, bv, Wo
    )
    res = run_spmd(in_maps)
    return combine_outputs(res.results, bo)
